# revision 5
# baseline (speedup 1.0000x reference)
"""Trainium2 Bass kernel for nn_CPLoss (connection/polygon/circle loss).

Strategy (8 NeuronCores, SPMD, data-parallel over conns/points/groups):
  Host stages planar field arrays (integer gather + layout + sign/abs bit
  tricks only); all floating-point arithmetic runs on device.

  Per-slot trig runs directly on ACT from fp8 angle planes:
      sin a = Sin(a)            (|a| < ~5 for N(0,1) angles -- in range)
      cos a = Sin(pi/2 - |a|)   (argument in [pi/2 - 5, pi/2] -- in range)
  |a| is staged as a separate fp8 plane (host bitmask, no FP math), which
  keeps both Sin arguments inside the accurate table range.

  Rotation + endpoint difference run on DVE in fp16 2x mode; the B
  endpoint's base coords are sign-flipped on the host so the difference
  is a pair-sum.  Translation terms (positions/base_offsets/centers
  composites) ride fp16 accumulate-DMA chains (gpsimd software DGE,
  AluOp.add) so they cost no compute-engine time.

  Work is balanced across engines: Pool takes the hinge/conn squares and
  the circle x^2+y^2 pair-sum; ACT takes trig, all sqrts, the circle
  squares, and the final square-accumulates; DVE keeps rotations, folds,
  and the circle-group segment-sum ladder.  The circle loss uses
      sum_g sum_k ((dc-avg)/avg)^2 = sum_g (64*Q_g/S_g^2) - 8*G
  (Q = sum dc^2, S = sum dc per group); -8*G is a host-side constant.

  ACT activation-table switches: 2 total (Sin set for both rounds' trig,
  then the Sqrt set for both rounds' tails).

  Output: per-core partial sums [128, 3*R] fp32; host combines in float64.
"""

import os
import sys

import numpy as np

sys.path.insert(0, "/opt/trn_rl_repo")

import concourse.mybir as mybir  # noqa: E402
import concourse.tile as tile  # noqa: E402
from concourse import bacc  # noqa: E402
from concourse.bass_utils import run_bass_kernel_spmd  # noqa: E402

F32 = mybir.dt.float32
F16 = mybir.dt.float16
F8 = mybir.dt.float8e4
ALU = mybir.AluOpType
ACTF = mybir.ActivationFunctionType

NC = 8
P_TOT = 2_000_000
K_PP = 4
N_TOT = P_TOT * K_PP
C_TOT = 2_000_000
G_TOT = 500_000
KC = 8
M_TOT = G_TOT * KC

C_C = C_TOT // NC            # 250_000 connections / core
G_C = G_TOT // NC            # 62_500 groups / core
M_C = M_TOT // NC            # 500_000 circle points / core

C_CP = 128 * 1968            # 251_904 padded conns
M_CP = 128 * 3936            # 503_808 padded circle points
G_CP = M_CP // KC            # 62_976 padded groups

ROUNDS = int(os.environ.get("KERNEL_ROUNDS", "2"))
CF = 1968 // ROUNDS          # conns per partition per round
MF = 3936 // ROUNDS          # circle points per partition per round
GF = MF // KC                # groups per partition per round

TRACE = os.environ.get("KERNEL_TRACE", "0") == "1"

PI_HALF = 1.5707963267948966


def _ts(i, n):
    return slice(i * n, (i + 1) * n)


def build_program():
    nc = bacc.Bacc("TRN2", target_bir_lowering=False, debug=False,
                   num_devices=NC, dynamic_dma_scratch_size=32768)

    # conn fp8 planes: 0: aA  1: aB  2: |aA|  3: |aB|
    cg8 = nc.dram_tensor("cg8", [4, C_CP], F8, kind="ExternalInput")
    # conn fp16 planes: 0: xA  1: -xB  2: yA  3: -yB  4: len
    #   5-20: T-chain, 4 term groups x [cTx, cTy, hTx, hTy]
    cg16 = nc.dram_tensor("cg16", [21, C_CP], F16, kind="ExternalInput")
    # circ fp8 planes: 0: a  1: |a|
    mg8 = nc.dram_tensor("mg8", [2, M_CP], F8, kind="ExternalInput")
    # circ fp16 planes: 0: x  1: y;  2-7: T-chain 3 groups x [Tx, Ty]
    mg16 = nc.dram_tensor("mg16", [8, M_CP], F16, kind="ExternalInput")
    out = nc.dram_tensor("partials", [128, 3 * ROUNDS], F32,
                         kind="ExternalOutput")

    def dview(t, p0, p1, sl, f):
        # planar DRAM slice [planes p0:p1, round window sl] as [128, p1-p0, f]
        return t[p0:p1, sl].rearrange("c (p f) -> p c f", p=128)

    with tile.TileContext(nc) as tc:
        with (
            tc.tile_pool(name="accp", bufs=1) as accp,
            tc.tile_pool(name="wp", bufs=1) as wp,
        ):
            acc = accp.tile([128, 3 * ROUNDS], F32)
            nc.vector.memset(acc[:], 0.0)
            consts = {}
            for name, val in [("zero", 0.0), ("one", 1.0),
                              ("pi_half", PI_HALF)]:
                t = accp.tile([128, 1], F32, tag="c_" + name)
                nc.vector.memset(t[:], val)
                consts[name] = t

            def stage_A(r):
                """All input DMAs for round r (front-loaded)."""
                csl = _ts(r, 128 * CF)
                msl = _ts(r, 128 * MF)
                raw8c = wp.tile([128, 4, CF], F8, tag="raw8c", bufs=2,
                                name="raw8c")
                nc.sync.dma_start(out=raw8c[:], in_=dview(cg8, 0, 4, csl, CF))
                raw16c = wp.tile([128, 5, CF], F16, tag="raw16c", bufs=2,
                                 name="raw16c")
                nc.sync.dma_start(out=raw16c[:],
                                  in_=dview(cg16, 0, 5, csl, CF))
                raw8m = wp.tile([128, 2, MF], F8, tag="raw8m", bufs=2,
                                name="raw8m")
                nc.sync.dma_start(out=raw8m[:], in_=dview(mg8, 0, 2, msl, MF))
                raw16m = wp.tile([128, 2, MF], F16, tag="raw16m", bufs=2,
                                 name="raw16m")
                nc.sync.dma_start(out=raw16m[:],
                                  in_=dview(mg16, 0, 2, msl, MF))
                # T chains: initial write via HWDGE, accумs via gpsimd SWDGE
                tch = wp.tile([128, 4, CF], F16, tag="tch", bufs=2,
                              name="tch")
                nc.sync.dma_start(out=tch[:], in_=dview(cg16, 5, 9, csl, CF))
                tcm = wp.tile([128, 2, MF], F16, tag="tcm", bufs=2,
                              name="tcm")
                nc.sync.dma_start(out=tcm[:], in_=dview(mg16, 2, 4, msl, MF))
                for p0 in (9, 13, 17):
                    nc.gpsimd.dma_start(out=tch[:],
                                        in_=dview(cg16, p0, p0 + 4, csl, CF),
                                        accum_op=ALU.add)
                for p0 in (4, 6):
                    nc.gpsimd.dma_start(out=tcm[:],
                                        in_=dview(mg16, p0, p0 + 2, msl, MF),
                                        accum_op=ALU.add)
                return raw8c, raw16c, raw8m, raw16m, tch, tcm

            def stage_B_trig(r, raw8c, raw8m):
                """ACT Sin-table block: sin/cos for both streams."""
                cs_c = wp.tile([128, 2, 2, CF], F16, tag="cs_c", bufs=2,
                               name="cs_c")
                nc.scalar.activation(
                    cs_c[:, 1, :, :].rearrange("p c f -> p (c f)"),
                    raw8c[:, 0:2, :].rearrange("p c f -> p (c f)"),
                    ACTF.Sin, bias=consts["zero"][:])
                nc.scalar.activation(
                    cs_c[:, 0, :, :].rearrange("p c f -> p (c f)"),
                    raw8c[:, 2:4, :].rearrange("p c f -> p (c f)"),
                    ACTF.Sin, bias=consts["pi_half"][:], scale=-1.0)
                cs_m = wp.tile([128, 2, MF], F16, tag="cs_m", bufs=2,
                               name="cs_m")
                nc.scalar.activation(cs_m[:, 1, :], raw8m[:, 0, :],
                                     ACTF.Sin, bias=consts["zero"][:])
                nc.scalar.activation(cs_m[:, 0, :], raw8m[:, 1, :],
                                     ACTF.Sin, bias=consts["pi_half"][:],
                                     scale=-1.0)
                return cs_c, cs_m

            def stage_B_rot(r, cs_c, cs_m, raw16c, raw16m, tch, tcm):
                """DVE rotations (+ translation folds), fp16 2x throughout."""
                co = cs_c[:, 0, :, :]
                si = cs_c[:, 1, :, :]
                x = raw16c[:, 0:2, :]
                y = raw16c[:, 2:4, :]
                ma = wp.tile([128, 2, 2, CF], F16, tag="ma", name="ma")
                mb = wp.tile([128, 2, 2, CF], F16, tag="mb", name="mb")
                nc.vector.tensor_mul(out=ma[:, 0, :, :], in0=co, in1=x)
                nc.vector.tensor_mul(out=ma[:, 1, :, :], in0=si, in1=y)
                # rx/ry fold back into ma (elementwise in-place is safe)
                nc.vector.tensor_sub(out=ma[:, 0, :, :], in0=ma[:, 0, :, :],
                                     in1=ma[:, 1, :, :])
                nc.vector.tensor_mul(out=mb[:, 0, :, :], in0=si, in1=x)
                nc.vector.tensor_mul(out=mb[:, 1, :, :], in0=co, in1=y)
                nc.vector.tensor_add(out=ma[:, 1, :, :], in0=mb[:, 0, :, :],
                                     in1=mb[:, 1, :, :])
                cd = wp.tile([128, 2, CF], F16, tag="cd", bufs=2, name="cd")
                nc.vector.tensor_add(out=cd[:], in0=ma[:, :, 0, :],
                                     in1=ma[:, :, 1, :])
                nc.vector.tensor_add(out=cd[:], in0=cd[:], in1=tch[:, 0:2, :])

                com = cs_m[:, 0, :]
                sim = cs_m[:, 1, :]
                xm = raw16m[:, 0, :]
                ym = raw16m[:, 1, :]
                mam = wp.tile([128, 2, MF], F16, tag="mam", name="mam")
                mbm = wp.tile([128, 2, MF], F16, tag="mbm", name="mbm")
                pc = wp.tile([128, 2, MF], F16, tag="pc", bufs=2, name="pc")
                nc.vector.tensor_mul(out=mam[:, 0, :], in0=com, in1=xm)
                nc.vector.tensor_mul(out=mam[:, 1, :], in0=sim, in1=ym)
                nc.vector.tensor_sub(out=pc[:, 0, :], in0=mam[:, 0, :],
                                     in1=mam[:, 1, :])
                nc.vector.tensor_mul(out=mbm[:, 0, :], in0=sim, in1=xm)
                nc.vector.tensor_mul(out=mbm[:, 1, :], in0=com, in1=ym)
                nc.vector.tensor_add(out=pc[:, 1, :], in0=mbm[:, 0, :],
                                     in1=mbm[:, 1, :])
                nc.vector.tensor_add(out=pc[:], in0=pc[:], in1=tcm[:])
                return cd, pc

            def stage_C(r, raw16c, tch, cd, pc):
                """Distance chains, reduces, loss accumulation."""
                # Pool: conn + hinge squares (in place), circle pair-sum
                nc.gpsimd.tensor_mul(out=cd[:], in0=cd[:], in1=cd[:])
                hd = tch[:, 2:4, :]
                hsq = wp.tile([128, 2, CF], F16, tag="hsq", name="hsq")
                nc.gpsimd.tensor_mul(out=hsq[:], in0=hd, in1=hd)
                hq = wp.tile([128, CF], F16, tag="hq", name="hq")
                nc.gpsimd.tensor_add(out=hq[:], in0=hsq[:, 0, :],
                                     in1=hsq[:, 1, :])

                # ACT: circle squares in place (pc -> pc^2)
                nc.scalar.activation(
                    pc[:].rearrange("p c f -> p (c f)"),
                    pc[:].rearrange("p c f -> p (c f)"),
                    ACTF.Square, bias=consts["zero"][:])
                # Pool: qd = px^2 + py^2 into pc[0]; dc goes to pc[1]
                nc.gpsimd.tensor_add(out=pc[:, 0, :], in0=pc[:, 0, :],
                                     in1=pc[:, 1, :])

                # DVE: cq = dx^2 + dy^2
                cq = wp.tile([128, CF], F16, tag="cq", name="cq")
                nc.vector.tensor_add(out=cq[:], in0=cd[:, 0, :],
                                     in1=cd[:, 1, :])

                # ---- Sqrt-table ACT block ---------------------------------
                nc.scalar.activation(pc[:, 1, :], pc[:, 0, :], ACTF.Sqrt,
                                     bias=consts["zero"][:])
                nc.scalar.activation(cq[:], cq[:], ACTF.Sqrt,
                                     bias=consts["zero"][:])
                ce = wp.tile([128, CF], F16, tag="ce", name="ce")
                nc.vector.tensor_sub(out=ce[:], in0=cq[:],
                                     in1=raw16c[:, 4, :])
                nc.scalar.activation(ce[:], ce[:], ACTF.Square,
                                     accum_out=acc[:, 3 * r:3 * r + 1])

                nc.scalar.activation(hq[:], hq[:], ACTF.Sqrt,
                                     bias=consts["zero"][:])
                nc.scalar.activation(hq[:], hq[:], ACTF.Relu,
                                     bias=consts["one"][:], scale=-1.0)
                nc.scalar.activation(hq[:], hq[:], ACTF.Square,
                                     accum_out=acc[:, 3 * r + 1:3 * r + 2])

                # DVE: fused Q|S group ladder ([2, GF, 8] -> [2, GF])
                qv = pc[:].rearrange("p c (g k) -> p c g k", k=KC)
                f4 = wp.tile([128, 2, GF, 4], F16, tag="f4", name="f4")
                f2 = wp.tile([128, 2, GF, 2], F16, tag="f2", name="f2")
                qs = wp.tile([128, 2, GF], F32, tag="qs", name="qs")
                nc.vector.tensor_add(out=f4[:], in0=qv[:, :, :, 0:4],
                                     in1=qv[:, :, :, 4:8])
                nc.vector.tensor_add(out=f2[:], in0=f4[:, :, :, 0:2],
                                     in1=f4[:, :, :, 2:4])
                nc.vector.tensor_add(out=qs[:], in0=f2[:, :, :, 0],
                                     in1=f2[:, :, :, 1])
                ss = wp.tile([128, GF], F32, tag="ss", name="ss")
                nc.vector.tensor_mul(out=ss[:], in0=qs[:, 1, :],
                                     in1=qs[:, 1, :])
                nc.vector.reciprocal_approx_fast(ss[:], ss[:])
                yv = wp.tile([128, GF], F32, tag="yv", name="yv")
                nc.vector.tensor_mul(out=yv[:], in0=qs[:, 0, :], in1=ss[:])
                nc.scalar.activation(yv[:], yv[:], ACTF.Identity,
                                     bias=consts["zero"][:], scale=64.0,
                                     accum_out=acc[:, 3 * r + 2:3 * r + 3])

            # warm the Sin table under the first DMAs
            warm = accp.tile([128, 1], F16, tag="warm")
            nc.scalar.activation(warm[:], consts["zero"][:], ACTF.Sin,
                                 bias=consts["zero"][:])
            raws = {r: stage_A(r) for r in range(ROUNDS)}
            trig = {}
            rots = {}
            for r in range(ROUNDS):
                trig[r] = stage_B_trig(r, raws[r][0], raws[r][2])
                rots[r] = stage_B_rot(r, *trig[r], raws[r][1], raws[r][3],
                                      raws[r][4], raws[r][5])
            for r in range(ROUNDS):
                stage_C(r, raws[r][1], raws[r][4], *rots[r])

            nc.sync.dma_start(out=out[:], in_=acc[:])

    nc.compile()
    return nc


_PROGRAM = None


def _get_program():
    global _PROGRAM
    if _PROGRAM is None:
        _PROGRAM = build_program()
    return _PROGRAM


def _negate16(a):
    # exact sign flip via bit manipulation (no FP arithmetic)
    b = np.ascontiguousarray(a, dtype=np.float16)
    v = b.view(np.uint16) ^ np.uint16(0x8000)
    return v.view(np.float16)


def _f8(a):
    import ml_dtypes
    return np.ascontiguousarray(a, dtype=np.float16).astype(
        ml_dtypes.float8_e4m3fn)


def _abs8(a8):
    # |a| via fp8 sign-bit clear (no FP arithmetic)
    return (a8.view(np.uint8) & np.uint8(0x7F)).view(a8.dtype)


def kernel(**inputs):
    positions = np.asarray(inputs["positions"], dtype=np.float16)
    angles8 = _f8(np.asarray(inputs["angles"], dtype=np.float16))
    circle_centers = np.asarray(inputs["circle_centers"], dtype=np.float16)
    base_points = np.asarray(inputs["base_points"], dtype=np.float16)
    base_offsets = np.asarray(inputs["base_offsets"], dtype=np.float16)
    connection_lengths = np.asarray(inputs["connection_lengths"],
                                    dtype=np.float16)
    connection_ids = np.asarray(inputs["connection_ids"]).astype(np.int64)
    connected_polys = np.asarray(inputs["connected_polys"]).astype(np.int64)
    circle_poly_ids = np.asarray(inputs["circle_poly_ids"]).astype(np.int64)
    poly_ids = np.asarray(inputs["poly_ids"]).astype(np.int64)
    grouping = np.asarray(inputs["circle_poly_grouping"]).astype(np.int64)

    assert grouping.shape == (M_TOT,) and np.array_equal(
        grouping, np.repeat(np.arange(G_TOT, dtype=np.int64), KC)
    ), "circle_poly_grouping must be repeat(arange(G), 8)"

    nc = _get_program()

    neg_pos = _negate16(positions)
    neg_off = _negate16(base_offsets)

    in_maps = []
    for c in range(NC):
        csl = _ts(c, C_C)
        msl = _ts(c, M_C)
        ia = connection_ids[csl, 0]
        ib = connection_ids[csl, 1]
        pa = poly_ids[ia]
        pb = poly_ids[ib]
        ha = connected_polys[csl, 0]
        hb = connected_polys[csl, 1]

        cg8p = np.zeros((4, C_CP), dtype=angles8.dtype)
        cg8p[0, :C_C] = angles8[pa]
        cg8p[1, :C_C] = angles8[pb]
        cg8p[2] = _abs8(cg8p[0])
        cg8p[3] = _abs8(cg8p[1])

        cg16p = np.zeros((21, C_CP), dtype=np.float16)
        cg16p[0, :C_C] = base_points[ia, 0]
        cg16p[1, :C_C] = _negate16(base_points[ib, 0])
        cg16p[2, :C_C] = base_points[ia, 1]
        cg16p[3, :C_C] = _negate16(base_points[ib, 1])
        cg16p[4, :C_C] = connection_lengths[csl]
        # T chain groups: [cTx, cTy, hTx, hTy] per term
        cg16p[5, :C_C] = positions[pa, 0]
        cg16p[6, :C_C] = positions[pa, 1]
        cg16p[7, :C_C] = positions[ha, 0]
        cg16p[8, :C_C] = positions[ha, 1]
        cg16p[9, :C_C] = base_offsets[pa, 0]
        cg16p[10, :C_C] = base_offsets[pa, 1]
        cg16p[11, :C_C] = base_offsets[ha, 0]
        cg16p[12, :C_C] = base_offsets[ha, 1]
        cg16p[13, :C_C] = neg_pos[pb, 0]
        cg16p[14, :C_C] = neg_pos[pb, 1]
        cg16p[15, :C_C] = neg_pos[hb, 0]
        cg16p[16, :C_C] = neg_pos[hb, 1]
        cg16p[17, :C_C] = neg_off[pb, 0]
        cg16p[18, :C_C] = neg_off[pb, 1]
        cg16p[19, :C_C] = neg_off[hb, 0]
        cg16p[20, :C_C] = neg_off[hb, 1]

        mi = circle_poly_ids[msl]
        mp = poly_ids[mi]
        gsl = _ts(c, G_C)
        mg8p = np.zeros((2, M_CP), dtype=angles8.dtype)
        mg8p[0, :M_C] = angles8[mp]
        mg8p[1] = _abs8(mg8p[0])

        mg16p = np.zeros((8, M_CP), dtype=np.float16)
        mg16p[0, :M_C] = base_points[mi, 0]
        mg16p[0, M_C:] = 1.0        # pad: point (1,0) -> dc=1, group term 0
        mg16p[1, :M_C] = base_points[mi, 1]
        mg16p[2, :M_C] = positions[mp, 0]
        mg16p[3, :M_C] = positions[mp, 1]
        mg16p[4, :M_C] = base_offsets[mp, 0]
        mg16p[5, :M_C] = base_offsets[mp, 1]
        mg16p[6, :M_C] = _negate16(np.repeat(circle_centers[gsl, 0], KC))
        mg16p[7, :M_C] = _negate16(np.repeat(circle_centers[gsl, 1], KC))

        in_maps.append({"cg8": cg8p, "cg16": cg16p,
                        "mg8": mg8p, "mg16": mg16p})

    try:
        res = run_bass_kernel_spmd(nc, in_maps, core_ids=list(range(NC)),
                                   trace=TRACE)
    except ModuleNotFoundError:
        res = run_bass_kernel_spmd(nc, in_maps, core_ids=list(range(NC)),
                                   trace=False)
    if TRACE and res.exec_time_ns is not None:
        print(f"HW exec time: {res.exec_time_ns} ns")

    conn = hinge = circ = 0.0
    for c in range(NC):
        p = res.results[c]["partials"].astype(np.float64)
        conn += p[:, 0::3].sum()
        hinge += p[:, 1::3].sum()
        circ += p[:, 2::3].sum()

    # hinge pads: T=0 -> pd=0 -> (1-0)^2 = 1 each
    hinge -= float((C_CP - C_C) * NC)
    # circle identity constant: sum_g (64 Q/S^2 - 8); pads net to 0
    circ -= 8.0 * G_CP * NC
    loss = conn + hinge + 50.0 * circ / float(M_TOT)
    return np.float32(loss)


# revision 12
# speedup vs baseline: 1.0755x; 1.0755x over previous
"""Trainium2 Bass kernel for nn_CPLoss (connection/polygon/circle loss).

Strategy (8 NeuronCores, SPMD, data-parallel over conns/points/groups):
  Host stages planar field arrays (integer gather + layout + sign/abs bit
  tricks only); all floating-point arithmetic runs on device.

  Per-slot trig runs directly on ACT from fp8 angle planes:
      sin a = Sin(a)            (|a| < ~5 for N(0,1) angles -- in range)
      cos a = Sin(pi/2 - |a|)   (argument in [pi/2 - 5, pi/2] -- in range)
  |a| is staged as a separate fp8 plane (host bitmask, no FP math), which
  keeps both Sin arguments inside the accurate table range.

  Rotation + endpoint difference run on DVE in fp16 2x mode; the B
  endpoint's base coords are sign-flipped on the host so the difference
  is a pair-sum.  Translation terms (positions/base_offsets/centers
  composites) ride fp16 accumulate-DMA chains (gpsimd software DGE,
  AluOp.add) so they cost no compute-engine time.

  Work is balanced across engines: Pool takes the hinge/conn squares and
  the circle x^2+y^2 pair-sum; ACT takes trig, all sqrts, the circle
  squares, and the final square-accumulates; DVE keeps rotations, folds,
  and the circle-group segment-sum ladder.  The circle loss uses
      sum_g sum_k ((dc-avg)/avg)^2 = sum_g (64*Q_g/S_g^2) - 8*G
  (Q = sum dc^2, S = sum dc per group); -8*G is a host-side constant.

  ACT activation-table switches: 2 total (Sin set for both rounds' trig,
  then the Sqrt set for both rounds' tails).

  Output: per-core partial sums [128, 3*R] fp32; host combines in float64.
"""

import os
import sys

import numpy as np

sys.path.insert(0, "/opt/trn_rl_repo")

import concourse.mybir as mybir  # noqa: E402
import concourse.tile as tile  # noqa: E402
from concourse import bacc  # noqa: E402
from concourse.bass_utils import run_bass_kernel_spmd  # noqa: E402

F32 = mybir.dt.float32
F16 = mybir.dt.float16
F8 = mybir.dt.float8e4
ALU = mybir.AluOpType
ACTF = mybir.ActivationFunctionType

NC = 8
P_TOT = 2_000_000
K_PP = 4
N_TOT = P_TOT * K_PP
C_TOT = 2_000_000
G_TOT = 500_000
KC = 8
M_TOT = G_TOT * KC

C_C = C_TOT // NC            # 250_000 connections / core
G_C = G_TOT // NC            # 62_500 groups / core
M_C = M_TOT // NC            # 500_000 circle points / core

C_CP = 128 * 1968            # 251_904 padded conns
M_CP = 128 * 3936            # 503_808 padded circle points
G_CP = M_CP // KC            # 62_976 padded groups

ROUNDS = int(os.environ.get("KERNEL_ROUNDS", "2"))
CF = 1968 // ROUNDS          # conns per partition per round
MF = 3936 // ROUNDS          # circle points per partition per round
GF = MF // KC                # groups per partition per round

TRACE = os.environ.get("KERNEL_TRACE", "0") == "1"

PI_HALF = 1.5707963267948966


def _ts(i, n):
    return slice(i * n, (i + 1) * n)


def build_program():
    nc = bacc.Bacc("TRN2", target_bir_lowering=False, debug=False,
                   num_devices=NC, dynamic_dma_scratch_size=32768)

    # conn fp8 planes: 0: aA  1: aB  2: |aA|  3: |aB|
    #   4-19: T-chain, 4 term groups x [cTx, cTy, hTx, hTy]
    cg8 = nc.dram_tensor("cg8", [20, C_CP], F8, kind="ExternalInput")
    # conn fp16 planes: 0: xA  1: -xB  2: yA  3: -yB  4: len
    cg16 = nc.dram_tensor("cg16", [5, C_CP], F16, kind="ExternalInput")
    # circ fp8 planes: 0: a  1: |a|;  2-7: T-chain 3 groups x [Tx, Ty]
    mg8 = nc.dram_tensor("mg8", [8, M_CP], F8, kind="ExternalInput")
    # circ fp16 planes: 0: x  1: y
    mg16 = nc.dram_tensor("mg16", [2, M_CP], F16, kind="ExternalInput")
    out = nc.dram_tensor("partials", [128, 3 * ROUNDS], F32,
                         kind="ExternalOutput")

    def dview(t, p0, p1, sl, f):
        # planar DRAM slice [planes p0:p1, round window sl] as [128, p1-p0, f]
        return t[p0:p1, sl].rearrange("c (p f) -> p c f", p=128)

    with tile.TileContext(nc) as tc:
        with (
            tc.tile_pool(name="accp", bufs=1) as accp,
            tc.tile_pool(name="wp", bufs=1) as wp,
        ):
            acc = accp.tile([128, 3 * ROUNDS], F32)
            nc.vector.memset(acc[:], 0.0)
            consts = {}
            for name, val in [("zero", 0.0), ("one", 1.0),
                              ("pi_half", PI_HALF)]:
                t = accp.tile([128, 1], F32, tag="c_" + name)
                nc.vector.memset(t[:], val)
                consts[name] = t

            def stage_A_raw(r):
                """Raw input DMAs for round r."""
                csl = _ts(r, 128 * CF)
                msl = _ts(r, 128 * MF)
                raw8c = wp.tile([128, 4, CF], F8, tag="raw8c", bufs=2,
                                name="raw8c")
                nc.sync.dma_start(out=raw8c[:], in_=dview(cg8, 0, 4, csl, CF))
                raw8m = wp.tile([128, 2, MF], F8, tag="raw8m", bufs=2,
                                name="raw8m")
                nc.sync.dma_start(out=raw8m[:], in_=dview(mg8, 0, 2, msl, MF))
                raw16c = wp.tile([128, 5, CF], F16, tag="raw16c", bufs=2,
                                 name="raw16c")
                nc.sync.dma_start(out=raw16c[:],
                                  in_=dview(cg16, 0, 5, csl, CF))
                raw16m = wp.tile([128, 2, MF], F16, tag="raw16m", bufs=2,
                                 name="raw16m")
                nc.sync.dma_start(out=raw16m[:],
                                  in_=dview(mg16, 0, 2, msl, MF))
                return raw8c, raw16c, raw8m, raw16m

            def stage_A_chains(r):
                """fp8 T-chains: initial write via HWDGE, accums via SWDGE."""
                csl = _ts(r, 128 * CF)
                msl = _ts(r, 128 * MF)
                tch = wp.tile([128, 4, CF], F8, tag="tch", bufs=2,
                              name="tch")
                nc.sync.dma_start(out=tch[:], in_=dview(cg8, 4, 8, csl, CF))
                tcm = wp.tile([128, 2, MF], F8, tag="tcm", bufs=2,
                              name="tcm")
                nc.sync.dma_start(out=tcm[:], in_=dview(mg8, 2, 4, msl, MF))
                for p0 in (8, 12, 16):
                    nc.gpsimd.dma_start(out=tch[:],
                                        in_=dview(cg8, p0, p0 + 4, csl, CF),
                                        accum_op=ALU.add)
                for p0 in (4, 6):
                    nc.gpsimd.dma_start(out=tcm[:],
                                        in_=dview(mg8, p0, p0 + 2, msl, MF),
                                        accum_op=ALU.add)
                return tch, tcm

            def stage_B_trig(r, raw8c, raw8m):
                """ACT Sin-table block: sin/cos for both streams."""
                cs_c = wp.tile([128, 2, 2, CF], F16, tag="cs_c", bufs=2,
                               name="cs_c")
                nc.scalar.activation(
                    cs_c[:, 1, :, :].rearrange("p c f -> p (c f)"),
                    raw8c[:, 0:2, :].rearrange("p c f -> p (c f)"),
                    ACTF.Sin, bias=consts["zero"][:])
                nc.scalar.activation(
                    cs_c[:, 0, :, :].rearrange("p c f -> p (c f)"),
                    raw8c[:, 2:4, :].rearrange("p c f -> p (c f)"),
                    ACTF.Sin, bias=consts["pi_half"][:], scale=-1.0)
                cs_m = wp.tile([128, 2, MF], F16, tag="cs_m", bufs=2,
                               name="cs_m")
                nc.scalar.activation(cs_m[:, 1, :], raw8m[:, 0, :],
                                     ACTF.Sin, bias=consts["zero"][:])
                nc.scalar.activation(cs_m[:, 0, :], raw8m[:, 1, :],
                                     ACTF.Sin, bias=consts["pi_half"][:],
                                     scale=-1.0)
                return cs_c, cs_m

            def stage_B_rot(r, cs_c, cs_m, raw16c, raw16m, tch, tcm):
                """DVE rotations (+ translation folds), fp16 2x throughout."""
                co = cs_c[:, 0, :, :]
                si = cs_c[:, 1, :, :]
                x = raw16c[:, 0:2, :]
                y = raw16c[:, 2:4, :]
                ma = wp.tile([128, 2, 2, CF], F16, tag="ma", name="ma")
                mb = wp.tile([128, 2, 2, CF], F16, tag="mb", name="mb")
                nc.vector.tensor_mul(out=ma[:, 0, :, :], in0=co, in1=x)
                nc.vector.tensor_mul(out=ma[:, 1, :, :], in0=si, in1=y)
                # rx/ry fold back into ma (elementwise in-place is safe)
                nc.vector.tensor_sub(out=ma[:, 0, :, :], in0=ma[:, 0, :, :],
                                     in1=ma[:, 1, :, :])
                nc.vector.tensor_mul(out=mb[:, 0, :, :], in0=si, in1=x)
                nc.vector.tensor_mul(out=mb[:, 1, :, :], in0=co, in1=y)
                nc.vector.tensor_add(out=ma[:, 1, :, :], in0=mb[:, 0, :, :],
                                     in1=mb[:, 1, :, :])
                cd = wp.tile([128, 2, CF], F16, tag="cd", bufs=2, name="cd")
                nc.vector.tensor_add(out=cd[:], in0=ma[:, :, 0, :],
                                     in1=ma[:, :, 1, :])
                nc.vector.tensor_add(out=cd[:], in0=cd[:], in1=tch[:, 0:2, :])

                com = cs_m[:, 0, :]
                sim = cs_m[:, 1, :]
                xm = raw16m[:, 0, :]
                ym = raw16m[:, 1, :]
                mam = wp.tile([128, 2, MF], F16, tag="mam", name="mam")
                mbm = wp.tile([128, 2, MF], F16, tag="mbm", name="mbm")
                pc = wp.tile([128, 2, MF], F16, tag="pc", bufs=2, name="pc")
                nc.vector.tensor_mul(out=mam[:, 0, :], in0=com, in1=xm)
                nc.vector.tensor_mul(out=mam[:, 1, :], in0=sim, in1=ym)
                nc.vector.tensor_sub(out=pc[:, 0, :], in0=mam[:, 0, :],
                                     in1=mam[:, 1, :])
                nc.vector.tensor_mul(out=mbm[:, 0, :], in0=sim, in1=xm)
                nc.vector.tensor_mul(out=mbm[:, 1, :], in0=com, in1=ym)
                nc.vector.tensor_add(out=pc[:, 1, :], in0=mbm[:, 0, :],
                                     in1=mbm[:, 1, :])
                nc.vector.tensor_add(out=pc[:], in0=pc[:], in1=tcm[:])
                return cd, pc

            def stage_C(r, raw16c, tch, cd, pc, last):
                """Distance chains, reduces, loss accumulation.  The last
                round's squares run on DVE (fast) since its tail is exposed;
                earlier rounds use Pool to keep DVE free."""
                sq_eng = nc.vector if last else nc.gpsimd
                # conn + hinge squares (conn in place), circle pair-sum
                sq_eng.tensor_mul(out=cd[:], in0=cd[:], in1=cd[:])
                hd = tch[:, 2:4, :]
                hsq = wp.tile([128, 2, CF], F16, tag="hsq", name="hsq")
                nc.gpsimd.tensor_mul(out=hsq[:], in0=hd, in1=hd)
                hq = wp.tile([128, CF], F16, tag="hq", name="hq")
                nc.gpsimd.tensor_add(out=hq[:], in0=hsq[:, 0, :],
                                     in1=hsq[:, 1, :])

                # ACT: circle squares in place (pc -> pc^2)
                nc.scalar.activation(
                    pc[:].rearrange("p c f -> p (c f)"),
                    pc[:].rearrange("p c f -> p (c f)"),
                    ACTF.Square, bias=consts["zero"][:])
                # qd = px^2 + py^2 into pc[0]; dc goes to pc[1]
                sq_eng.tensor_add(out=pc[:, 0, :], in0=pc[:, 0, :],
                                  in1=pc[:, 1, :])

                # DVE: cq = dx^2 + dy^2
                cq = wp.tile([128, CF], F16, tag="cq", name="cq")
                nc.vector.tensor_add(out=cq[:], in0=cd[:, 0, :],
                                     in1=cd[:, 1, :])

                # ---- Sqrt-table ACT block ---------------------------------
                nc.scalar.activation(pc[:, 1, :], pc[:, 0, :], ACTF.Sqrt,
                                     bias=consts["zero"][:])
                nc.scalar.activation(cq[:], cq[:], ACTF.Sqrt,
                                     bias=consts["zero"][:])
                ce = wp.tile([128, CF], F16, tag="ce", name="ce")
                nc.vector.tensor_sub(out=ce[:], in0=cq[:],
                                     in1=raw16c[:, 4, :])
                nc.scalar.activation(ce[:], ce[:], ACTF.Square,
                                     accum_out=acc[:, 3 * r:3 * r + 1])

                nc.scalar.activation(hq[:], hq[:], ACTF.Sqrt,
                                     bias=consts["zero"][:])
                nc.scalar.activation(hq[:], hq[:], ACTF.Relu,
                                     bias=consts["one"][:], scale=-1.0)
                nc.scalar.activation(hq[:], hq[:], ACTF.Square,
                                     accum_out=acc[:, 3 * r + 1:3 * r + 2])

                # DVE: fused Q|S group ladder ([2, GF, 8] -> [2, GF])
                qv = pc[:].rearrange("p c (g k) -> p c g k", k=KC)
                f4 = wp.tile([128, 2, GF, 4], F16, tag="f4", name="f4")
                f2 = wp.tile([128, 2, GF, 2], F16, tag="f2", name="f2")
                qs = wp.tile([128, 2, GF], F32, tag="qs", name="qs")
                nc.vector.tensor_add(out=f4[:], in0=qv[:, :, :, 0:4],
                                     in1=qv[:, :, :, 4:8])
                nc.vector.tensor_add(out=f2[:], in0=f4[:, :, :, 0:2],
                                     in1=f4[:, :, :, 2:4])
                nc.vector.tensor_add(out=qs[:], in0=f2[:, :, :, 0],
                                     in1=f2[:, :, :, 1])
                ss = wp.tile([128, GF], F32, tag="ss", name="ss")
                nc.vector.tensor_mul(out=ss[:], in0=qs[:, 1, :],
                                     in1=qs[:, 1, :])
                nc.vector.reciprocal_approx_fast(ss[:], ss[:])
                yv = wp.tile([128, GF], F32, tag="yv", name="yv")
                nc.vector.tensor_mul(out=yv[:], in0=qs[:, 0, :], in1=ss[:])
                nc.scalar.activation(yv[:], yv[:], ACTF.Identity,
                                     bias=consts["zero"][:], scale=64.0,
                                     accum_out=acc[:, 3 * r + 2:3 * r + 3])

            # warm the Sin table under the first DMAs
            warm = accp.tile([128, 1], F16, tag="warm")
            nc.scalar.activation(warm[:], consts["zero"][:], ACTF.Sin,
                                 bias=consts["zero"][:])
            # emission order: round-r chains beat round-(r+1) syncs into
            # the DMA queue; all trig precedes all sqrt-table ACT ops.
            raws = {}
            chains = {}
            trig = {}
            rots = {}
            raws[0] = stage_A_raw(0)
            chains[0] = stage_A_chains(0)
            trig[0] = stage_B_trig(0, raws[0][0], raws[0][2])
            for r in range(1, ROUNDS):
                raws[r] = stage_A_raw(r)
                chains[r] = stage_A_chains(r)
                rots[r - 1] = stage_B_rot(r - 1, *trig[r - 1],
                                          raws[r - 1][1], raws[r - 1][3],
                                          *chains[r - 1])
                trig[r] = stage_B_trig(r, raws[r][0], raws[r][2])
            rl = ROUNDS - 1
            rots[rl] = stage_B_rot(rl, *trig[rl], raws[rl][1], raws[rl][3],
                                   *chains[rl])
            for r in range(ROUNDS):
                stage_C(r, raws[r][1], chains[r][0], *rots[r],
                        last=(r == ROUNDS - 1))

            nc.sync.dma_start(out=out[:], in_=acc[:])

    nc.compile()
    return nc


_PROGRAM = None


def _get_program():
    global _PROGRAM
    if _PROGRAM is None:
        _PROGRAM = build_program()
    return _PROGRAM


def _negate16(a):
    # exact sign flip via bit manipulation (no FP arithmetic)
    b = np.ascontiguousarray(a, dtype=np.float16)
    v = b.view(np.uint16) ^ np.uint16(0x8000)
    return v.view(np.float16)


def _f8(a):
    import ml_dtypes
    return np.ascontiguousarray(a, dtype=np.float16).astype(
        ml_dtypes.float8_e4m3fn)


def _abs8(a8):
    # |a| via fp8 sign-bit clear (no FP arithmetic)
    return (a8.view(np.uint8) & np.uint8(0x7F)).view(a8.dtype)


def _neg8(a8):
    # exact fp8 sign flip via bit manipulation (no FP arithmetic)
    return (a8.view(np.uint8) ^ np.uint8(0x80)).view(a8.dtype)


def kernel(**inputs):
    positions = np.asarray(inputs["positions"], dtype=np.float16)
    angles8 = _f8(np.asarray(inputs["angles"], dtype=np.float16))
    circle_centers = np.asarray(inputs["circle_centers"], dtype=np.float16)
    base_points = np.asarray(inputs["base_points"], dtype=np.float16)
    base_offsets = np.asarray(inputs["base_offsets"], dtype=np.float16)
    connection_lengths = np.asarray(inputs["connection_lengths"],
                                    dtype=np.float16)
    connection_ids = np.asarray(inputs["connection_ids"]).astype(np.int64)
    connected_polys = np.asarray(inputs["connected_polys"]).astype(np.int64)
    circle_poly_ids = np.asarray(inputs["circle_poly_ids"]).astype(np.int64)
    poly_ids = np.asarray(inputs["poly_ids"]).astype(np.int64)
    grouping = np.asarray(inputs["circle_poly_grouping"]).astype(np.int64)

    assert grouping.shape == (M_TOT,) and np.array_equal(
        grouping, np.repeat(np.arange(G_TOT, dtype=np.int64), KC)
    ), "circle_poly_grouping must be repeat(arange(G), 8)"

    nc = _get_program()

    pos8 = _f8(positions)
    off8 = _f8(base_offsets)
    neg_pos8 = _neg8(pos8)
    neg_off8 = _neg8(off8)
    cen8 = _f8(circle_centers)

    in_maps = []
    for c in range(NC):
        csl = _ts(c, C_C)
        msl = _ts(c, M_C)
        ia = connection_ids[csl, 0]
        ib = connection_ids[csl, 1]
        pa = poly_ids[ia]
        pb = poly_ids[ib]
        ha = connected_polys[csl, 0]
        hb = connected_polys[csl, 1]

        cg8p = np.zeros((20, C_CP), dtype=angles8.dtype)
        cg8p[0, :C_C] = angles8[pa]
        cg8p[1, :C_C] = angles8[pb]
        cg8p[2] = _abs8(cg8p[0])
        cg8p[3] = _abs8(cg8p[1])
        # T chain groups: [cTx, cTy, hTx, hTy] per term
        cg8p[4, :C_C] = pos8[pa, 0]
        cg8p[5, :C_C] = pos8[pa, 1]
        cg8p[6, :C_C] = pos8[ha, 0]
        cg8p[7, :C_C] = pos8[ha, 1]
        cg8p[8, :C_C] = off8[pa, 0]
        cg8p[9, :C_C] = off8[pa, 1]
        cg8p[10, :C_C] = off8[ha, 0]
        cg8p[11, :C_C] = off8[ha, 1]
        cg8p[12, :C_C] = neg_pos8[pb, 0]
        cg8p[13, :C_C] = neg_pos8[pb, 1]
        cg8p[14, :C_C] = neg_pos8[hb, 0]
        cg8p[15, :C_C] = neg_pos8[hb, 1]
        cg8p[16, :C_C] = neg_off8[pb, 0]
        cg8p[17, :C_C] = neg_off8[pb, 1]
        cg8p[18, :C_C] = neg_off8[hb, 0]
        cg8p[19, :C_C] = neg_off8[hb, 1]

        cg16p = np.zeros((5, C_CP), dtype=np.float16)
        cg16p[0, :C_C] = base_points[ia, 0]
        cg16p[1, :C_C] = _negate16(base_points[ib, 0])
        cg16p[2, :C_C] = base_points[ia, 1]
        cg16p[3, :C_C] = _negate16(base_points[ib, 1])
        cg16p[4, :C_C] = connection_lengths[csl]

        mi = circle_poly_ids[msl]
        mp = poly_ids[mi]
        gsl = _ts(c, G_C)
        mg8p = np.zeros((8, M_CP), dtype=angles8.dtype)
        mg8p[0, :M_C] = angles8[mp]
        mg8p[1] = _abs8(mg8p[0])
        mg8p[2, :M_C] = pos8[mp, 0]
        mg8p[3, :M_C] = pos8[mp, 1]
        mg8p[4, :M_C] = off8[mp, 0]
        mg8p[5, :M_C] = off8[mp, 1]
        mg8p[6, :M_C] = _neg8(np.repeat(cen8[gsl, 0], KC))
        mg8p[7, :M_C] = _neg8(np.repeat(cen8[gsl, 1], KC))

        mg16p = np.zeros((2, M_CP), dtype=np.float16)
        mg16p[0, :M_C] = base_points[mi, 0]
        mg16p[0, M_C:] = 1.0        # pad: point (1,0) -> dc=1, group term 0
        mg16p[1, :M_C] = base_points[mi, 1]

        in_maps.append({"cg8": cg8p, "cg16": cg16p,
                        "mg8": mg8p, "mg16": mg16p})

    try:
        res = run_bass_kernel_spmd(nc, in_maps, core_ids=list(range(NC)),
                                   trace=TRACE)
    except ModuleNotFoundError:
        res = run_bass_kernel_spmd(nc, in_maps, core_ids=list(range(NC)),
                                   trace=False)
    if TRACE and res.exec_time_ns is not None:
        print(f"HW exec time: {res.exec_time_ns} ns")

    conn = hinge = circ = 0.0
    for c in range(NC):
        p = res.results[c]["partials"].astype(np.float64)
        conn += p[:, 0::3].sum()
        hinge += p[:, 1::3].sum()
        circ += p[:, 2::3].sum()

    # hinge pads: T=0 -> pd=0 -> (1-0)^2 = 1 each
    hinge -= float((C_CP - C_C) * NC)
    # circle identity constant: sum_g (64 Q/S^2 - 8); pads net to 0
    circ -= 8.0 * G_CP * NC
    loss = conn + hinge + 50.0 * circ / float(M_TOT)
    return np.float32(loss)
